# revision 1
# baseline (speedup 1.0000x reference)
"""Transformer block (BatchedPoincareBlock) Bass kernel for 8 TRN2 cores.

Sharding: Megatron head-split attention (2 heads/core, LN1 replicated, fused
LN->transpose->QKV), one AllToAll to re-shard token-wise, then
sequence-parallel AO + LN2 + MLP on 512 own tokens/core with full weights.
Softmax is max-free (scores bounded ~+-60 here): P = exp(s) unnormalized,
denominator via a ones column appended to the PV lhsT, divide at eviction.
"""
import sys
for p in ('/opt/trn_rl_repo', '/root/.axon_site/_ro/trn_rl_repo'):
    if p not in sys.path:
        sys.path.insert(0, p)
import numpy as np
import concourse.bass as bass
import concourse.mybir as mybir
import concourse.tile as tile
import concourse.bacc as bacc
from concourse.masks import make_identity

F32 = mybir.dt.float32
F32R = mybir.dt.float32r
BF16 = mybir.dt.bfloat16
AF = mybir.ActivationFunctionType

NC = 8
B, S, E, H = 2, 2048, 1024, 16
HD = E // H          # 64
DFF = 4 * E          # 4096
T = B * S            # 4096
TC = T // NC         # 512 own tokens
LN_EPS = 1e-5
NEG = -1e30


def np_dt(dt):
    if dt == BF16:
        import ml_dtypes
        return ml_dtypes.bfloat16
    return np.float32


class Cfg:
    def __init__(self, dt_qkv=F32R, dt_att=F32R, dt_ao=BF16, dt_mlp=BF16,
                 reps=1, skip_trivial=True, use_hw_gelu=True, debug=False,
                 no_comm=False, phases="ABCDE"):
        self.dt_qkv = dt_qkv
        self.dt_att = dt_att
        self.dt_ao = dt_ao
        self.dt_mlp = dt_mlp
        self.reps = reps
        self.skip_trivial = skip_trivial
        self.use_hw_gelu = use_hw_gelu
        self.debug = debug
        self.no_comm = no_comm
        self.phases = phases
        self.key = (str(dt_qkv), str(dt_att), str(dt_ao), str(dt_mlp),
                    reps, skip_trivial, use_hw_gelu, debug, no_comm,
                    phases)


def build_program(cfg):
    c = cfg
    nc = bacc.Bacc(None, target_bir_lowering=False)

    x_in = nc.dram_tensor("x", [T, E], F32, kind="ExternalInput")
    xown_in = nc.dram_tensor("x_own", [TC, E], F32, kind="ExternalInput")
    wqkvT_in = nc.dram_tensor("wqkvT", [E, 384], c.dt_qkv, kind="ExternalInput")
    bqkv_in = nc.dram_tensor("bqkv", [128, 3], F32, kind="ExternalInput")
    waoT_in = nc.dram_tensor("waoT", [E, E], c.dt_ao, kind="ExternalInput")
    bao_in = nc.dram_tensor("bao", [1, E], F32, kind="ExternalInput")
    wfcT_in = nc.dram_tensor("wfcT", [E, DFF], c.dt_mlp, kind="ExternalInput")
    bfc_in = nc.dram_tensor("bfc", [128, 32], F32, kind="ExternalInput")
    wpoT_in = nc.dram_tensor("wpoT", [DFF, E], c.dt_mlp, kind="ExternalInput")
    bpo_in = nc.dram_tensor("bpo", [1, E], F32, kind="ExternalInput")
    ln1g_in = nc.dram_tensor("ln1g", [128, 8], F32, kind="ExternalInput")
    ln1b_in = nc.dram_tensor("ln1b", [128, 8], F32, kind="ExternalInput")
    ln2g_in = nc.dram_tensor("ln2g", [128, 8], F32, kind="ExternalInput")
    ln2b_in = nc.dram_tensor("ln2b", [128, 8], F32, kind="ExternalInput")
    negtri_in = nc.dram_tensor("negtri", [128, 128], F32, kind="ExternalInput")
    out_ext = nc.dram_tensor("out_own", [TC, E], F32, kind="ExternalOutput")
    if c.debug:
        dbg_q = nc.dram_tensor("dbg_q", [128, T], F32, kind="ExternalOutput")
        dbg_k = nc.dram_tensor("dbg_k", [128, T], F32, kind="ExternalOutput")
        dbg_at = nc.dram_tensor("dbg_at", [128, T], F32, kind="ExternalOutput")
        dbg_x2 = nc.dram_tensor("dbg_x2", [TC, E], F32, kind="ExternalOutput")
        dbg_h2T = nc.dram_tensor("dbg_h2T", [E, TC], F32, kind="ExternalOutput")
        dbg_g = nc.dram_tensor("dbg_g", [128, TC], F32, kind="ExternalOutput")

    skip = c.skip_trivial

    with tile.TileContext(nc) as tc:
        with (
            tc.tile_pool(name="consts", bufs=1) as consts,
            tc.tile_pool(name="wpool", bufs=1) as wpool,
            tc.tile_pool(name="stats", bufs=8) as stats_p,
            tc.tile_pool(name="small", bufs=4) as small,
            tc.tile_pool(name="dram", bufs=1, space="DRAM") as dram,
        ):
            ident_f32 = consts.tile([128, 128], F32, name="ident_f32",
                                    tag="ident_f32")
            make_identity(nc, ident_f32[:])
            ident = {}
            for dt in {c.dt_qkv, c.dt_att, c.dt_mlp}:
                if dt == F32:
                    ident[dt] = ident_f32
                    continue
                idt = consts.tile([128, 128], dt, name=f"ident_{dt}",
                                  tag=f"ident_{dt}")
                nc.vector.tensor_copy(idt[:], ident_f32[:])
                ident[dt] = idt
            negtri = consts.tile([128, 128], F32, name="negtri", tag="negtri")
            nc.sync.dma_start(negtri[:], negtri_in[:])
            eps_t = consts.tile([128, 1], F32, name="eps_t", tag="eps_t")
            nc.vector.memset(eps_t[:], LN_EPS)
            ones64_f = consts.tile([1, 64], F32, name="ones64_f", tag="ones64_f")
            nc.vector.memset(ones64_f[:], 1.0)
            ones64 = consts.tile([1, 64], c.dt_att, name="ones64", tag="ones64")
            nc.vector.tensor_copy(ones64[:], ones64_f[:])
            if not skip:
                ones128f = consts.tile([1, 128], F32, name="ones128f",
                                       tag="ones128f")
                nc.vector.memset(ones128f[:], 1.0)
                ones128 = consts.tile([1, 128], c.dt_mlp, name="ones128",
                                      tag="ones128")
                nc.vector.tensor_copy(ones128[:], ones128f[:])
            vpc_f = consts.tile([128, 2], F32, name="vpc_f", tag="vpc_f")
            nc.vector.memset(vpc_f[:, 0:1], 1.0)
            nc.vector.memset(vpc_f[:, 1:2], 0.0)
            vpcols = consts.tile([128, 2], c.dt_att, name="vpcols", tag="vpcols")
            nc.vector.tensor_copy(vpcols[:], vpc_f[:])
            z384_f = consts.tile([128, 384], F32, name="z384_f", tag="z384_f")
            nc.vector.memset(z384_f[:], 0.0)
            zeros384 = consts.tile([128, 384], c.dt_att, name="zeros384",
                                   tag="zeros384")
            nc.vector.tensor_copy(zeros384[:], z384_f[:])
            bqkv = consts.tile([128, 3], F32, name="bqkv", tag="bqkv")
            bfc = consts.tile([128, 32], F32, name="bfc", tag="bfc")
            bao = consts.tile([1, E], F32, name="bao", tag="bao")
            bpo = consts.tile([1, E], F32, name="bpo", tag="bpo")
            ln = {}
            if not skip:
                nc.sync.dma_start(bqkv[:], bqkv_in[:])
                nc.sync.dma_start(bfc[:], bfc_in[:])
                nc.sync.dma_start(bao[:], bao_in[:])
                nc.sync.dma_start(bpo[:], bpo_in[:])
                bpo_r = consts.tile([1, E], c.dt_mlp, name="bpo_r", tag="bpo_r")
                nc.vector.tensor_copy(bpo_r[:], bpo[:])
                for nm, src in (("ln1g", ln1g_in), ("ln1b", ln1b_in),
                                ("ln2g", ln2g_in), ("ln2b", ln2b_in)):
                    t = consts.tile([128, 8], F32, name=nm, tag=nm)
                    nc.sync.dma_start(t[:], src[:])
                    ln[nm] = t

            wqkvT = []
            for k in range(8):
                t = wpool.tile([128, 384], c.dt_qkv, name=f"wqkvT{k}",
                               tag=f"wqkvT{k}")
                nc.sync.dma_start(t[:], wqkvT_in[128 * k:128 * (k + 1), :])
                wqkvT.append(t)
            a2a_in = dram.tile([E, TC], c.dt_ao, name="a2a_in", tag="a2a_in")
            a2a_out = dram.tile([E, TC], c.dt_ao, name="a2a_out", tag="a2a_out")

            def ln_stats(x_tile, tag):
                st = stats_p.tile([128, 2, 6], F32, name=f"st_{tag}", tag="st")
                xv = x_tile[:].rearrange("p (n f) -> p n f", n=2)
                nc.vector.bn_stats(st[:, 0, :], xv[:, 0, :])
                nc.vector.bn_stats(st[:, 1, :], xv[:, 1, :])
                mv = stats_p.tile([128, 2], F32, name=f"mv_{tag}", tag="mv")
                nc.vector.bn_aggr(mv[:], st[:])
                rs = stats_p.tile([128, 1], F32, name=f"rs_{tag}", tag="rs")
                nc.scalar.activation(rs[:], mv[:, 1:2], AF.Sqrt, bias=eps_t[:])
                nc.vector.reciprocal(rs[:], rs[:])
                nmr = stats_p.tile([128, 1], F32, name=f"nmr_{tag}", tag="nmr")
                nc.vector.tensor_scalar(nmr[:], mv[:, 0:1], rs[:], -1.0,
                                        op0=mybir.AluOpType.mult,
                                        op1=mybir.AluOpType.mult)
                return rs, nmr

            for rep in range(c.reps):
                # ============ Phases A-C (attention) ============
                with (
                    tc.tile_pool(name="bigAC", bufs=1) as big,
                    tc.tile_pool(name="sbA", bufs=1) as sbA,
                    tc.tile_pool(name="sbB", bufs=1) as sbB,
                ):
                    qkvT = [big.tile([128, T], c.dt_att, name=f"qkvT{m}",
                                     tag=f"qkvT{m}") for m in range(3)]
                    attnT = big.tile([128, T], c.dt_ao, name="attnT", tag="attnT")

                    # ---- Phase A: LN1 + h^T + QKV ----
                    with tc.tile_pool(name="psA", bufs=1, space="PSUM") as psA:
                        for nb in range(8):
                            hn = []
                            for tt in range(4):
                                xt = sbA.tile([128, E], F32, name=f"x_{nb}_{tt}",
                                              tag="xt", bufs=3)
                                r0 = (nb * 4 + tt) * 128
                                nc.sync.dma_start(xt[:], x_in[r0:r0 + 128, :])
                                rs, nmr = ln_stats(xt, f"a{nb}{tt}")
                                h = sbA.tile([128, E], c.dt_qkv,
                                             name=f"hn_{nb}_{tt}", tag="hn", bufs=6)
                                nc.scalar.activation(h[:], xt[:], AF.Identity,
                                                     bias=nmr[:], scale=rs[:])
                                hn.append(h)
                            hT = []
                            for e in range(8):
                                ptr = psA.tile([128, 512], c.dt_qkv,
                                               name=f"ptr_{nb}_{e}", tag="ptr",
                                               bufs=3)
                                for tt in range(4):
                                    nc.tensor.transpose(
                                        ptr[:, tt * 128:(tt + 1) * 128],
                                        hn[tt][:, e * 128:(e + 1) * 128],
                                        ident[c.dt_qkv][:])
                                ht = sbA.tile([128, 512], c.dt_qkv,
                                              name=f"hT_{nb}_{e}", tag=f"hT{e}",
                                              bufs=3)
                                if skip:
                                    if e % 2 == 0:
                                        nc.scalar.copy(ht[:], ptr[:])
                                    else:
                                        nc.vector.tensor_copy(ht[:], ptr[:])
                                else:
                                    # y = g*hT + b, per-partition in hT layout
                                    nc.scalar.activation(
                                        ht[:], ptr[:], AF.Identity,
                                        bias=ln["ln1b"][:, e:e + 1],
                                        scale=ln["ln1g"][:, e:e + 1])
                                hT.append(ht)
                            for m in range(3):
                                pq = psA.tile([128, 512], F32, name=f"pq_{nb}_{m}",
                                              tag="pq", bufs=3)
                                for k in range(8):
                                    nc.tensor.matmul(
                                        pq[:], wqkvT[k][:, m * 128:(m + 1) * 128],
                                        hT[k][:], start=(k == 0), stop=(k == 7))
                                dst = qkvT[m][:, nb * 512:(nb + 1) * 512]
                                if skip:
                                    nc.scalar.copy(dst, pq[:])
                                else:
                                    nc.scalar.activation(dst, pq[:], AF.Identity,
                                                         bias=bqkv[:, m:m + 1])

                    qT, kTt, vT = qkvT

                    # ---- Phase B: attention ----
                    with tc.tile_pool(name="psB", bufs=1, space="PSUM") as psB:
                        for b in range(2 if "B" in c.phases else 0):
                            c0 = b * S
                            vh = [[None] * 16, [None] * 16]
                            for kt in range(16):
                                ptv = psB.tile([128, 128], c.dt_att,
                                               name=f"ptv_{b}_{kt}", tag="ptv",
                                               bufs=2)
                                nc.tensor.transpose(
                                    ptv[:],
                                    vT[:, c0 + kt * 128:c0 + (kt + 1) * 128],
                                    ident[c.dt_att][:])
                                for h in range(2):
                                    vt = sbB.tile([128, 66], c.dt_att,
                                                  name=f"vp_{b}_{h}_{kt}",
                                                  tag=f"vp{h}_{kt}")
                                    nc.vector.tensor_copy(
                                        vt[:, 0:64], ptv[:, 64 * h:64 * h + 64])
                                    nc.vector.tensor_copy(vt[:, 64:66], vpcols[:])
                                    vh[h][kt] = vt
                            for h in range(2):
                                rq = 64 * h
                                for J in range(4):
                                    ppv = psB.tile([66, 512], F32,
                                                   name=f"ppv_{b}_{h}_{J}",
                                                   tag="ppv", bufs=2)
                                    nkt = 4 * J + 4
                                    Ps = []
                                    for kt in range(nkt):
                                        cc = kt - 4 * J
                                        off = 128 * cc if cc > 0 else 0
                                        ps = psB.tile([128, 512], F32,
                                                      name=f"ps_{b}_{h}_{J}_{kt}",
                                                      tag="ps", bufs=3)
                                        nc.tensor.matmul(
                                            ps[:, off:512],
                                            kTt[rq:rq + 64,
                                                c0 + kt * 128:c0 + (kt + 1) * 128],
                                            qT[rq:rq + 64,
                                               c0 + J * 512 + off:c0 + (J + 1) * 512],
                                            start=True, stop=True)
                                        if cc >= 0:
                                            nc.vector.tensor_add(
                                                ps[:, 128 * cc:128 * (cc + 1)],
                                                ps[:, 128 * cc:128 * (cc + 1)],
                                                negtri[:])
                                        P = sbB.tile([128, 512], c.dt_att,
                                                     name=f"P_{b}_{h}_{J}_{kt}",
                                                     tag="P", bufs=8)
                                        if off > 0:
                                            nc.vector.tensor_copy(
                                                P[:, 0:off], zeros384[:, 0:off])
                                        nc.scalar.activation(P[:, off:512],
                                                             ps[:, off:512], AF.Exp)
                                        Ps.append(P)
                                    for kt in range(nkt):
                                        nc.tensor.matmul(
                                            ppv[:], vh[h][kt][:], Ps[kt][:],
                                            start=(kt == 0), stop=(kt == nkt - 1))
                                    rec = small.tile([1, 512], c.dt_att,
                                                     name=f"rec_{b}_{h}_{J}",
                                                     tag="rec")
                                    with nc.allow_low_precision(
                                            reason="f32r recip, 2^-12 rel ok"):
                                        nc.vector.reciprocal(rec[:],
                                                             ppv[64:65, :])
                                    recb = psB.tile([64, 512], F32,
                                                    name=f"recb_{b}_{h}_{J}",
                                                    tag="recb", bufs=1)
                                    nc.tensor.matmul(recb[:], ones64[:], rec[:],
                                                     start=True, stop=True)
                                    recs = small.tile([64, 512], F32,
                                                      name=f"recs_{b}_{h}_{J}",
                                                      tag="recs", bufs=2)
                                    nc.scalar.copy(recs[:], recb[:])
                                    nc.vector.tensor_mul(
                                        attnT[rq:rq + 64,
                                              c0 + J * 512:c0 + (J + 1) * 512],
                                        ppv[0:64, :], recs[:])

                    if "C" not in c.phases:
                        continue
                    if c.debug:
                        def _dump(dst, t, dt):
                            if dt == BF16:
                                nc.gpsimd.dma_start(dst[:], t[:])
                            else:
                                nc.sync.dma_start(dst[:], t[:].bitcast(F32))
                        _dump(dbg_q, qT, c.dt_att)
                        _dump(dbg_k, kTt, c.dt_att)
                        _dump(dbg_at, attnT, c.dt_ao)
                    # ---- Phase C: AllToAll ----
                    for j in range(8):
                        nc.sync.dma_start(a2a_in[128 * j:128 * (j + 1), :],
                                          attnT[:, 512 * j:512 * (j + 1)])
                    if c.no_comm:
                        nc.sync.dma_start(a2a_out[:], a2a_in[:])
                    else:
                        nc.gpsimd.collective_compute(
                            "AllToAll", mybir.AluOpType.bypass,
                            replica_groups=[list(range(NC))],
                            ins=[a2a_in.opt()], outs=[a2a_out.opt()],
                        )

                # ============ Phases D-E (own tokens) ============
                with (
                    tc.tile_pool(name="sbD", bufs=1) as sbD,
                    tc.tile_pool(name="sbE", bufs=1) as sbE,
                ):
                    if "D" not in c.phases:
                        continue
                    waoT = []
                    for k in range(8):
                        t = sbD.tile([128, E], c.dt_ao, name=f"waoT{k}",
                                     tag=f"waoT{k}")
                        nc.sync.dma_start(t[:], waoT_in[128 * k:128 * (k + 1), :])
                        waoT.append(t)
                    aol = []
                    for k in range(8):
                        t = sbD.tile([128, TC], c.dt_ao, name=f"aol{k}",
                                     tag=f"aol{k}")
                        nc.sync.dma_start(t[:], a2a_out[128 * k:128 * (k + 1), :])
                        aol.append(t)
                    xo = []
                    for m in range(4):
                        t = sbD.tile([128, E], F32, name=f"xo{m}", tag=f"xo{m}")
                        nc.sync.dma_start(t[:], xown_in[128 * m:128 * (m + 1), :])
                        xo.append(t)

                    # ---- Phase D: AO + residual + LN2 + h2T ----
                    x2 = []
                    h2n = []
                    with tc.tile_pool(name="psD", bufs=1, space="PSUM") as psD:
                        for m in range(4):
                            x2m = sbD.tile([128, E], F32, name=f"x2_{m}",
                                           tag=f"x2{m}")
                            for n in range(2):
                                pa = psD.tile([128, 512], F32, name=f"pao_{m}_{n}",
                                              tag="pao", bufs=2)
                                for k in range(8):
                                    nc.tensor.matmul(
                                        pa[:], aol[k][:, m * 128:(m + 1) * 128],
                                        waoT[k][:, n * 512:(n + 1) * 512],
                                        start=(k == 0), stop=(k == 7))
                                nc.vector.tensor_add(
                                    x2m[:, n * 512:(n + 1) * 512], pa[:],
                                    xo[m][:, n * 512:(n + 1) * 512])

                            x2.append(x2m)
                            if c.debug:
                                nc.sync.dma_start(
                                    dbg_x2[128 * m:128 * (m + 1), :], x2m[:])
                            rs, nmr = ln_stats(x2m, f"d{m}")
                            h = sbD.tile([128, E], c.dt_mlp, name=f"h2n_{m}",
                                         tag=f"h2n{m}")
                            nc.scalar.activation(h[:], x2m[:], AF.Identity,
                                                 bias=nmr[:], scale=rs[:])
                            h2n.append(h)
                        h2T = []
                        for e in range(8):
                            ptr = psD.tile([128, 512], c.dt_mlp, name=f"ptr2_{e}",
                                           tag="ptr2", bufs=2)
                            for tt in range(4):
                                nc.tensor.transpose(
                                    ptr[:, tt * 128:(tt + 1) * 128],
                                    h2n[tt][:, e * 128:(e + 1) * 128],
                                    ident[c.dt_mlp][:])
                            ht = sbD.tile([128, 512], c.dt_mlp, name=f"h2T_{e}",
                                          tag=f"h2T{e}")
                            if skip:
                                if e % 2 == 0:
                                    nc.scalar.copy(ht[:], ptr[:])
                                else:
                                    nc.vector.tensor_copy(ht[:], ptr[:])
                            else:
                                nc.scalar.activation(ht[:], ptr[:], AF.Identity,
                                                     bias=ln["ln2b"][:, e:e + 1],
                                                     scale=ln["ln2g"][:, e:e + 1])
                            if c.debug:
                                nc.gpsimd.dma_start(
                                    dbg_h2T[128 * e:128 * (e + 1), :], ht[:])
                            h2T.append(ht)

                    # ---- Phase E: MLP ----
                    if "E" not in c.phases:
                        continue
                    gT = [sbE.tile([128, TC], c.dt_mlp, name=f"gT{m}",
                                   tag=f"gT{m}") for m in range(32)]
                    with tc.tile_pool(name="psE", bufs=1, space="PSUM") as psE:
                        for half in range(2):
                            wf = []
                            for k in range(8):
                                t = sbE.tile([128, 2048], c.dt_mlp,
                                             name=f"wf_{half}_{k}", tag=f"wf{k}")
                                nc.sync.dma_start(
                                    t[:, 0:1024],
                                    wfcT_in[128 * k:128 * (k + 1),
                                            half * 2048:half * 2048 + 1024])
                                nc.sync.dma_start(
                                    t[:, 1024:2048],
                                    wfcT_in[128 * k:128 * (k + 1),
                                            half * 2048 + 1024:(half + 1) * 2048])
                                wf.append(t)
                            for mm in range(16):
                                m = half * 16 + mm
                                pf = psE.tile([128, 512], F32, name=f"pf_{m}",
                                              tag="pf", bufs=3)
                                for k in range(8):
                                    nc.tensor.matmul(
                                        pf[:], wf[k][:, mm * 128:(mm + 1) * 128],
                                        h2T[k][:], start=(k == 0), stop=(k == 7))
                                bias_ap = None if skip else bfc[:, m:m + 1]
                                if c.use_hw_gelu:
                                    if bias_ap is None:
                                        nc.scalar.activation(gT[m][:], pf[:],
                                                             AF.Gelu)
                                    else:
                                        nc.scalar.activation(gT[m][:], pf[:],
                                                             AF.Gelu, bias=bias_ap)
                                else:
                                    emit_tanh_gelu(nc, small, gT[m], pf,
                                                   bias_ap, m)

                            # PO: 4 accumulators live across all 32 ff-chunks,
                            # split by E-half (n) outside
                        ppo_t = [psE.tile([128, 512], F32, name=f"ppo_{m}",
                                          tag=f"ppo{m}", bufs=1) for m in range(4)]
                        if c.debug:
                            nc.gpsimd.dma_start(dbg_g[:], gT[0][:])
                        outb = [sbE.tile([128, E], F32, name=f"outb{m}",
                                         tag=f"outb{m}") for m in range(4)]
                        for n in range(2):
                            for kk in range(32):
                                wp = sbE.tile([128, 512], c.dt_mlp,
                                              name=f"wp_{n}_{kk}", tag="wp",
                                              bufs=6)
                                nc.sync.dma_start(
                                    wp[:], wpoT_in[128 * kk:128 * (kk + 1),
                                                   n * 512:(n + 1) * 512])
                                for m in range(4):
                                    nc.tensor.matmul(
                                        ppo_t[m][:],
                                        gT[kk][:, m * 128:(m + 1) * 128],
                                        wp[:], start=(kk == 0),
                                        stop=(kk == 31 and skip))
                                if kk == 31 and not skip:
                                    for m in range(4):
                                        nc.tensor.matmul(
                                            ppo_t[m][:], ones128[:],
                                            bpo_r[:, n * 512:(n + 1) * 512],
                                            start=False, stop=True)
                            for m in range(4):
                                nc.vector.tensor_add(
                                    outb[m][:, n * 512:(n + 1) * 512],
                                    ppo_t[m][:], x2[m][:, n * 512:(n + 1) * 512])

                        for m in range(4):
                            nc.sync.dma_start(out_ext[128 * m:128 * (m + 1), :],
                                              outb[m][:])
    nc.compile()
    return nc


def emit_tanh_gelu(nc, small, out_t, pf, bias_ap, m):
    """Exact GPT-2 gelu_new: 0.5x(1+tanh(sqrt(2/pi)(x+0.044715x^3)))."""
    xf = small.tile([128, 512], F32, name=f"g_x_{m}", tag="g_x")
    if bias_ap is None:
        nc.scalar.copy(xf[:], pf[:])
    else:
        nc.scalar.activation(xf[:], pf[:], AF.Identity, bias=bias_ap)
    s = small.tile([128, 512], F32, name=f"g_s_{m}", tag="g_s")
    nc.scalar.activation(s[:], xf[:], AF.Square)
    nc.vector.tensor_scalar(s[:], s[:], 0.044715, 1.0,
                            op0=mybir.AluOpType.mult, op1=mybir.AluOpType.add)
    nc.vector.tensor_mul(s[:], s[:], xf[:])
    th = small.tile([128, 512], F32, name=f"g_t_{m}", tag="g_t")
    nc.scalar.activation(th[:], s[:], AF.Tanh, scale=0.7978845608028654)
    nc.vector.tensor_mul(th[:], th[:], xf[:])
    nc.vector.tensor_add(th[:], th[:], xf[:])
    nc.scalar.activation(out_t[:], th[:], AF.Copy, scale=0.5)


# ======================= host side =======================

def prep_inputs(core, inputs, cfg):
    c = cfg
    x = np.ascontiguousarray(np.asarray(inputs["x"], np.float32).reshape(T, E))
    w_qkv = np.asarray(inputs["w_qkv"], np.float32)
    b_qkv = np.asarray(inputs["b_qkv"], np.float32)
    # reference layout: qkv.reshape(B,S,H,3*HD) -> head h rows are
    # w_qkv[192h:192h+64]=q, [+64:+128]=k, [+128:+192]=v
    hs = [2 * core, 2 * core + 1]
    wq = np.concatenate([w_qkv[192 * h:192 * h + 64] for h in hs]) * 0.125
    wk = np.concatenate([w_qkv[192 * h + 64:192 * h + 128] for h in hs])
    wv = np.concatenate([w_qkv[192 * h + 128:192 * h + 192] for h in hs])
    wqkvT = np.concatenate([wq, wk, wv], axis=0).T.copy()
    bq = np.concatenate([b_qkv[192 * h:192 * h + 64] for h in hs]) * 0.125
    bk = np.concatenate([b_qkv[192 * h + 64:192 * h + 128] for h in hs])
    bv = np.concatenate([b_qkv[192 * h + 128:192 * h + 192] for h in hs])
    negtri = np.where(np.arange(128)[:, None] > np.arange(128)[None, :],
                      np.float32(NEG), np.float32(0.0)).astype(np.float32)
    return {
        "x": x,
        "x_own": (x[core * TC:(core + 1) * TC]
                  + np.asarray(inputs["b_ao"], np.float32)[None, :]).copy(),
        "wqkvT": wqkvT.astype(np_dt(c.dt_qkv)),
        "bqkv": np.stack([bq, bk, bv], axis=1).copy(),
        "waoT": np.asarray(inputs["w_ao"], np.float32).T.copy().astype(np_dt(c.dt_ao)),
        "bao": np.asarray(inputs["b_ao"], np.float32).reshape(1, E).copy(),
        "wfcT": np.asarray(inputs["w_fc"], np.float32).T.copy().astype(np_dt(c.dt_mlp)),
        "bfc": np.asarray(inputs["b_fc"], np.float32).reshape(32, 128).T.copy(),
        "wpoT": np.asarray(inputs["w_po"], np.float32).T.copy().astype(np_dt(c.dt_mlp)),
        "bpo": np.asarray(inputs["b_po"], np.float32).reshape(1, E).copy(),
        "ln1g": np.asarray(inputs["ln1_g"], np.float32).reshape(8, 128).T.copy(),
        "ln1b": np.asarray(inputs["ln1_b"], np.float32).reshape(8, 128).T.copy(),
        "ln2g": np.asarray(inputs["ln2_g"], np.float32).reshape(8, 128).T.copy(),
        "ln2b": np.asarray(inputs["ln2_b"], np.float32).reshape(8, 128).T.copy(),
        "negtri": negtri,
    }


def check_trivial(inputs):
    z = lambda a: bool(np.all(np.asarray(a) == 0))
    o = lambda a: bool(np.all(np.asarray(a) == 1))
    return (z(inputs["b_qkv"]) and z(inputs["b_ao"]) and z(inputs["b_fc"])
            and z(inputs["b_po"]) and o(inputs["ln1_g"]) and z(inputs["ln1_b"])
            and o(inputs["ln2_g"]) and z(inputs["ln2_b"]))


_prog_cache = {}


def get_program(cfg):
    if cfg.key not in _prog_cache:
        _prog_cache[cfg.key] = build_program(cfg)
    return _prog_cache[cfg.key]


def run_block(inputs, cfg=None):
    from concourse.bass_utils import run_bass_kernel_spmd
    if cfg is None:
        cfg = Cfg(skip_trivial=check_trivial(inputs))
    nc = get_program(cfg)
    in_maps = [prep_inputs(cc, inputs, cfg) for cc in range(NC)]
    res = run_bass_kernel_spmd(nc, in_maps, list(range(NC)))
    out = np.concatenate([res.results[cc]["out_own"] for cc in range(NC)], axis=0)
    return out.reshape(B, S, E)


def kernel(**inputs):
    """Full-input entry point: takes the problem's full tensors, returns [B,S,E]."""
    return np.asarray(run_block(inputs), np.float32)

